# revision 12
# baseline (speedup 1.0000x reference)
"""Trainium2 Bass kernel for fused MHA block (nn_MHA_50895362457886).

Reference computation (per batch b):
  qkv = x @ W_in.T + b_in                      # [S, 3072]
  qkv = causal_depthwise_conv1d(qkv, conv_w) + conv_b
  q, k, v = split(qkv)  (16 q heads, 4 kv heads, head_dim 128, GQA rep 4)
  q, k = rotary(q), rotary(k)
  attn = softmax(causal(q @ k.T / sqrt(128)))
  ctx = attn @ v                               # [S, 16*128]
  out = ctx @ W_out.T + b_out                  # [S, 2048]

Sharding over 8 cores: core c -> (batch b = c//4, kv-group g = c%4).
Each core owns q heads 4g..4g+3 (512 channels), kv head g (128+128 channels),
computes attention for those heads and a partial out-projection against
W_out[:, 512g:512(g+1)].  Host sums the 4 partials per batch and adds b_out.

Everything on-device is computed in transposed layout (channels on
partitions, sequence on the free dim); matmuls run as float32r (full PE
rate for free-dim >= 256).
"""

import os
import numpy as np
from contextlib import ExitStack

import concourse.bass as bass
import concourse.bacc as bacc
import concourse.tile as tile
from concourse import mybir
from concourse.bass_utils import run_bass_kernel_spmd

F32 = mybir.dt.float32
F32R = mybir.dt.float32r
AOP = mybir.AluOpType
AFT = mybir.ActivationFunctionType

S = 2048          # sequence length
E = 2048          # embed dim
D = 128           # head dim
NQH = 4           # q heads per core
NF = 6            # channel tiles of 128 per core: 4 q heads, 1 k head, 1 v head
FTOT = NF * 128   # 768 channels per core
DCONV = 4
P = 128
ST_N = 4          # s tiles of 512 (projection)
KT_N = 16         # contraction tiles of 128 over E
QT_N = 4          # q tiles of 512 (attention)
KB_N = 16         # k blocks of 128 (attention)

_prog_cache = {}


def build_program():
    """Build the Bass program (same SPMD program for all 8 cores)."""
    if "nc" in _prog_cache:
        return _prog_cache["nc"]

    nc = bacc.Bacc("TRN2", target_bir_lowering=False, debug=False, num_devices=8)

    # ------- DRAM I/O -------
    xt = nc.dram_tensor("xt", [E, S], F32R, kind="ExternalInput").ap()
    w_in_t = nc.dram_tensor("w_in_t", [E, FTOT], F32R, kind="ExternalInput").ap()
    b_in_v = nc.dram_tensor("b_in_v", [FTOT, 1], F32, kind="ExternalInput").ap()
    conv_w_v = nc.dram_tensor("conv_w_v", [FTOT, DCONV], F32, kind="ExternalInput").ap()
    conv_b_v = nc.dram_tensor("conv_b_v", [FTOT, 1], F32, kind="ExternalInput").ap()
    w_out_t = nc.dram_tensor("w_out_t", [NQH * D, E], F32R, kind="ExternalInput").ap()
    cos_q = nc.dram_tensor("cos_q", [D, S], F32, kind="ExternalInput").ap()
    sin_q = nc.dram_tensor("sin_q", [D, S], F32, kind="ExternalInput").ap()
    cos_k = nc.dram_tensor("cos_k", [D, S], F32, kind="ExternalInput").ap()
    sin_k = nc.dram_tensor("sin_k", [D, S], F32, kind="ExternalInput").ap()
    ident = nc.dram_tensor("ident", [P, P], F32, kind="ExternalInput").ap()
    ones_f = nc.dram_tensor("ones_f", [P, P], F32, kind="ExternalInput").ap()
    iota_in = nc.dram_tensor("iota_in", [P, 512], F32, kind="ExternalInput").ap()
    thr_in = nc.dram_tensor("thr_in", [P, 4], F32, kind="ExternalInput").ap()
    out_t = nc.dram_tensor("out_t", [E, S], F32, kind="ExternalOutput").ap()

    with TileKernel(nc) as tk:
        tk.emit(xt, w_in_t, b_in_v, conv_w_v, conv_b_v, w_out_t,
                cos_q, sin_q, cos_k, sin_k, ident, ones_f, iota_in, thr_in, out_t)

    nc.compile()
    _prog_cache["nc"] = nc
    return nc


class TileKernel:
    def __init__(self, nc):
        self.nc = nc
        self.tc = tile.TileContext(nc)
        self.stack = ExitStack()

    def __enter__(self):
        self.tc.__enter__()
        self.stack.__enter__()
        return self

    def __exit__(self, *exc):
        self.stack.__exit__(*exc)
        return self.tc.__exit__(*exc)

    def emit(self, xt, w_in_t, b_in_v, conv_w_v, conv_b_v, w_out_t,
             cos_q, sin_q, cos_k, sin_k, ident, ones_f, iota_in, thr_in, out_t):
        nc, tc, ctx = self.nc, self.tc, self.stack

        # ---------- constants ----------
        const = ctx.enter_context(tc.tile_pool(name="const", bufs=1, side="right"))
        ident_t = const.tile([P, P], F32, name="ident_t")
        nc.sync.dma_start(ident_t[:], ident[:])
        ones_f_t = const.tile([P, P], F32, name="ones_f_t")
        nc.sync.dma_start(ones_f_t[:], ones_f[:])
        iota_t = const.tile([P, 512], F32, name="iota_t")
        nc.sync.dma_start(iota_t[:], iota_in[:])
        thr_t = const.tile([P, 4], F32, name="thr_t")
        nc.sync.dma_start(thr_t[:], thr_in[:])
        convw_t = const.tile([P, NF * DCONV], F32, name="convw_t")
        nc.sync.dma_start(
            convw_t[:].rearrange("p (f c) -> p f c", c=DCONV),
            conv_w_v.rearrange("(f p) c -> p f c", p=P))
        convb_t = const.tile([P, NF], F32, name="convb_t")
        nc.sync.dma_start(
            convb_t[:].rearrange("p (f o) -> p f o", o=1),
            conv_b_v.rearrange("(f p) o -> p f o", p=P))
        bin_t = const.tile([P, NF], F32, name="bin_t")
        nc.sync.dma_start(
            bin_t[:].rearrange("p (f o) -> p f o", o=1),
            b_in_v.rearrange("(f p) o -> p f o", p=P))

        # qkv_pre tiles (post-projection incl. b_in, 3 zero pad cols on left)
        qkv_scope = ExitStack()
        qkv_pool = qkv_scope.enter_context(tc.tile_pool(name="qkv", bufs=1))
        qkv_pad = [qkv_pool.tile([P, S + DCONV - 1], F32, name=f"qkv_pad{f}")
                   for f in range(NF)]

        # ---------- phase A: qkv = W @ x^T (+ b_in) ----------
        with tc.tile_pool(name="wA", bufs=1) as wA_pool, \
             tc.tile_pool(name="xA", bufs=2) as xA_pool, \
             tc.tile_pool(name="psA", bufs=3, space="PSUM") as psA:
            w_all = wA_pool.tile([P, KT_N * FTOT], F32R, name="w_all")
            nc.sync.dma_start(
                w_all[:].rearrange("p (kt f) -> p kt f", f=FTOT),
                w_in_t.rearrange("(kt p) f -> p kt f", p=P))
            for st in range(ST_N):
                xts = xA_pool.tile([P, KT_N * 512], F32R, name="xts", tag="xts")
                nc.sync.dma_start(
                    xts[:].rearrange("p (kt s) -> p kt s", s=512),
                    xt.rearrange("(kt p) s -> p kt s", p=P)[:, :, st * 512:(st + 1) * 512])
                for f in range(NF):
                    ps = psA.tile([P, 512], F32, name="psA_t", tag="psA_t")
                    for kt in range(KT_N):
                        nc.tensor.matmul(
                            ps[:],
                            w_all[:, kt * FTOT + f * P: kt * FTOT + (f + 1) * P],
                            xts[:, kt * 512:(kt + 1) * 512],
                            start=(kt == 0), stop=(kt == KT_N - 1))
                    # evacuate psum + add b_in
                    nc.vector.tensor_scalar(
                        qkv_pad[f][:, DCONV - 1 + st * 512: DCONV - 1 + (st + 1) * 512],
                        ps[:], bin_t[:, f:f + 1], None, op0=AOP.add)

        # conv+rotary outputs (QT/KT/VT layout: [128 ch, S]); opened after
        # the phase-A pools release their SBUF
        qc_pool = ctx.enter_context(tc.tile_pool(name="qc", bufs=1, side="right"))
        qc = [qc_pool.tile([P, S], F32R if f <= NQH else F32, name=f"qc{f}")
              for f in range(NF)]

        # ---------- conv + rotary ----------
        with tc.tile_pool(name="rope", bufs=1) as rope, \
             tc.tile_pool(name="sw", bufs=2) as sw_pool:
            cs_tiles = {}
            for nm, src in (("cq", cos_q), ("sq", sin_q), ("ck", cos_k), ("sk", sin_k)):
                t = rope.tile([D, S], F32, name=f"rope_{nm}", tag=f"rope_{nm}")
                nc.sync.dma_start(t[:], src[:])
                cs_tiles[nm] = t
            for f in range(NF):
                nc.vector.memset(qkv_pad[f][:, 0:DCONV - 1], 0.0)
                # depthwise conv: cv = sum_j w_j * qkv_pad[f][:, j:j+S] + conv_b
                cv = qc[f] if f == NQH + 1 else None
                if cv is None:
                    cv = sw_pool.tile([P, S], F32, name="cv", tag="cv")
                nc.vector.tensor_scalar(
                    cv[:], qkv_pad[f][:, 0:S],
                    convw_t[:, f * DCONV: f * DCONV + 1],
                    convb_t[:, f:f + 1],
                    op0=AOP.mult, op1=AOP.add)
                for j in range(1, DCONV):
                    nc.vector.scalar_tensor_tensor(
                        cv[:], qkv_pad[f][:, j:j + S],
                        convw_t[:, f * DCONV + j: f * DCONV + j + 1],
                        cv[:], op0=AOP.mult, op1=AOP.add)
                if f < NQH + 1:  # rotary on q heads and k head -> F32R qc[f]
                    cs = cs_tiles["cq"] if f < NQH else cs_tiles["ck"]
                    sn = cs_tiles["sq"] if f < NQH else cs_tiles["sk"]
                    sw = sw_pool.tile([P, S], F32, name="sw", tag="sw")
                    half = D // 2
                    nc.sync.dma_start(sw[0:half, :], cv[half:D, :])
                    nc.sync.dma_start(sw[half:D, :], cv[0:half, :])
                    nc.vector.tensor_mul(sw[:], sw[:], sn[:])
                    nc.vector.tensor_mul(cv[:], cv[:], cs[:])
                    nc.vector.tensor_add(qc[f][:], cv[:], sw[:])
        qkv_scope.close()

        # ---------- transpose V to [s, d] blocks ----------
        vsd_pool = ctx.enter_context(tc.tile_pool(name="vsd", bufs=1, side="right"))
        v_sd = vsd_pool.tile([P, KB_N * D], F32R, name="v_sd")
        with tc.tile_pool(name="psT", bufs=2, space="PSUM") as psT:
            for kb in range(KB_N):
                pt = psT.tile([P, P], F32, name="psT_t", tag="psT_t")
                nc.tensor.transpose(pt[:], qc[NQH + 1][:, kb * P:(kb + 1) * P], ident_t[:])
                nc.vector.tensor_copy(v_sd[:, kb * D:(kb + 1) * D], pt[:])

        # ---------- attention ----------
        ctx_pool = ctx.enter_context(tc.tile_pool(name="ctxu", bufs=1, side="right"))
        ctx_u = [ctx_pool.tile([P, S], F32R, name=f"ctx_u{h}") for h in range(NQH)]

        with tc.tile_pool(name="pstrip", bufs=2) as pstrip_pool, \
             tc.tile_pool(name="acc", bufs=2) as acc_pool, \
             tc.tile_pool(name="rcb", bufs=2) as rcb_pool, \
             tc.tile_pool(name="psS", bufs=2, space="PSUM") as psS, \
             tc.tile_pool(name="psCtx", bufs=2, space="PSUM") as psCtx, \
             tc.tile_pool(name="psDen", bufs=2, space="PSUM") as psDen:
            for h in range(NQH):
                for qt in range(QT_N):
                    nkb = 4 * (qt + 1)
                    strip = pstrip_pool.tile([P, nkb * 512], F32R, name="strip", tag="strip")
                    # scores^T chunks: psum pairs -> exp -> sbuf strip
                    for pair in range(nkb // 2):
                        ps = psS.tile([P, 1024], F32, name="psS_t", tag="psS_t")
                        for i in range(2):
                            kb = pair * 2 + i
                            nc.tensor.matmul(
                                ps[:, i * 512:(i + 1) * 512],
                                qc[NQH][:, kb * P:(kb + 1) * P],   # K^T block
                                qc[h][:, qt * 512:(qt + 1) * 512],  # Q^T
                                start=True, stop=True)
                        nc.scalar.activation(
                            strip[:, pair * 1024:(pair + 1) * 1024], ps[:], AFT.Exp)
                    # causal mask on diagonal chunks: keep where
                    # iota_q >= p + 128*(kb-4qt), else zero
                    for kb in range(4 * qt, nkb):
                        j = kb - 4 * qt
                        nc.vector.scalar_tensor_tensor(
                            strip[:, kb * 512:(kb + 1) * 512],
                            iota_t[:], thr_t[:, j:j + 1],
                            strip[:, kb * 512:(kb + 1) * 512],
                            op0=AOP.is_ge, op1=AOP.mult)
                    # denominator: acc[k_loc, q] = sum_kb strip, then
                    # replicated partition-sum via ones matmul, reciprocal
                    acc = acc_pool.tile([P, 512], F32, name="acc", tag="acc")
                    nc.vector.tensor_reduce(
                        acc[:],
                        strip[:].rearrange("p (kb q) -> p q kb", q=512),
                        axis=mybir.AxisListType.X, op=AOP.add)
                    dn = psDen.tile([P, 512], F32, name="dn", tag="dn")
                    nc.tensor.matmul(dn[:], ones_f_t[:], acc[:],
                                     start=True, stop=True)
                    rcb = rcb_pool.tile([P, 512], F32, name="rcb", tag="rcb")
                    nc.vector.reciprocal(rcb[:], dn[:])
                    # ctx^T += V^T_blk.T @ strip_blk
                    pc = psCtx.tile([P, 512], F32, name="psCtx_t", tag="psCtx_t")
                    for kb in range(nkb):
                        nc.tensor.matmul(
                            pc[:],
                            v_sd[:, kb * D:(kb + 1) * D],
                            strip[:, kb * 512:(kb + 1) * 512],
                            start=(kb == 0), stop=(kb == nkb - 1))
                    # evacuate ctx with softmax normalization fused in
                    nc.vector.tensor_mul(
                        ctx_u[h][:, qt * 512:(qt + 1) * 512], rcb[:], pc[:])

        # ---------- phase C: out^T = W_out_sh^T @ ctx ----------
        with tc.tile_pool(name="wC", bufs=1) as wC_pool, \
             tc.tile_pool(name="oC", bufs=3) as oC_pool, \
             tc.tile_pool(name="psC", bufs=3, space="PSUM") as psC:
            w_all_c = wC_pool.tile([P, NQH * E], F32R, name="w_all_c")
            nc.sync.dma_start(
                w_all_c[:].rearrange("p (ft e) -> p ft e", e=E),
                w_out_t.rearrange("(ft p) e -> p ft e", p=P))
            for et in range(E // P):
                for st in range(ST_N):
                    ps = psC.tile([P, 512], F32, name="psC_t", tag="psC_t")
                    for ft in range(NQH):
                        nc.tensor.matmul(
                            ps[:],
                            w_all_c[:, ft * E + et * P: ft * E + (et + 1) * P],
                            ctx_u[ft][:, st * 512:(st + 1) * 512],
                            start=(ft == 0), stop=(ft == NQH - 1))
                    ob = oC_pool.tile([P, 512], F32, name="ob", tag="ob")
                    nc.vector.tensor_copy(ob[:], ps[:])
                    nc.sync.dma_start(
                        out_t[et * P:(et + 1) * P, st * 512:(st + 1) * 512], ob[:])


def make_inputs_for_core(c, x, W_in, b_in, conv_w, conv_b, W_out):
    """Host-side sharding for core c (b = c//4, g = c%4)."""
    b, g = divmod(c, 4)
    rows = np.concatenate([
        np.arange(512 * g, 512 * (g + 1)),
        np.arange(2048 + 128 * g, 2048 + 128 * (g + 1)),
        np.arange(2560 + 128 * g, 2560 + 128 * (g + 1)),
    ])
    t = np.arange(S, dtype=np.float32)
    inv_freq = 1.0 / (10000.0 ** (np.arange(0, D, 2, dtype=np.float32) / D))
    freqs = np.outer(t, inv_freq)                    # [S, 64]
    cos = np.cos(freqs).T.astype(np.float32)         # [64, S]
    sin = np.sin(freqs).T.astype(np.float32)
    cos_full = np.concatenate([cos, cos], 0)         # [128, S]
    sin_signed = np.concatenate([-sin, sin], 0)
    scale = np.float32(1.0 / np.sqrt(D))
    return {
        "xt": np.ascontiguousarray(x[b].T).astype(np.float32),
        "w_in_t": np.ascontiguousarray(W_in[rows].T).astype(np.float32),
        "b_in_v": np.ascontiguousarray(b_in[rows]).reshape(FTOT, 1).astype(np.float32),
        "conv_w_v": np.ascontiguousarray(conv_w[rows]).astype(np.float32),
        "conv_b_v": np.ascontiguousarray(conv_b[rows]).reshape(FTOT, 1).astype(np.float32),
        "w_out_t": np.ascontiguousarray(W_out[:, 512 * g:512 * (g + 1)].T).astype(np.float32),
        "cos_q": cos_full, "sin_q": sin_signed,
        "cos_k": cos_full * scale, "sin_k": sin_signed * scale,
        "ident": np.eye(P, dtype=np.float32),
        "ones_f": np.ones((P, P), np.float32),
        "iota_in": np.broadcast_to(np.arange(512, dtype=np.float32), (P, 512)).copy(),
        "thr_in": (np.arange(P, dtype=np.float32)[:, None]
                   + 128.0 * np.arange(4, dtype=np.float32)[None, :]).copy(),
    }


def kernel(x, W_in, b_in, conv_w, conv_b, W_out, b_out):
    x = np.asarray(x); W_in = np.asarray(W_in); b_in = np.asarray(b_in)
    conv_w = np.asarray(conv_w); conv_b = np.asarray(conv_b)
    W_out = np.asarray(W_out); b_out = np.asarray(b_out)

    nc = build_program()
    in_maps = [make_inputs_for_core(c, x, W_in, b_in, conv_w, conv_b, W_out)
               for c in range(8)]
    trace = os.environ.get("KERNEL_TRACE", "0") == "1"
    res = run_bass_kernel_spmd(nc, in_maps, core_ids=list(range(8)), trace=trace)
    if trace:
        kernel.last_exec_time_ns = res.exec_time_ns
        kernel.last_profile = res
    B = x.shape[0]
    out = np.empty((B, S, E), np.float32)
    for b in range(B):
        acc = res.results[4 * b]["out_t"].astype(np.float32).copy()
        for g in range(1, 4):
            acc += res.results[4 * b + g]["out_t"]
        out[b] = acc.T + b_out[None, :]
    return out
